# revision 13
# baseline (speedup 1.0000x reference)
"""Trainium2 Bass kernel for a top-2-of-8 MoE layer (attention-pooled gating).

Strategy
--------
The reference computes every expert densely and combines with weights ``g``
that have exactly K=2 nonzeros per batch (softmax -> top-k mask -> renorm).
So the mathematically identical computation is: route each batch to its top-2
experts and compute only those 64 (batch, expert) pairs.

Host side (cheap, O(B*S*D)): attention-pool gating in fp32 mirroring the
reference op-for-op, top-2 selection, renormalized weights.

Scheduling: the 64 pairs are decomposed into 16 mono-expert parts of 3 pairs
and 16 mono-expert parts of 1 pair (always feasible: sum(floor(n_e/3)) >=
(64 - 2*8)/3 = 16), and every core gets two 3-parts and two 1-parts -- a
uniform [3, 3, 1, 1] slot pattern, so one SPMD program serves any routing
while loading each slot's expert weights ONCE (16.8 MB/core instead of the
33.6 MB/core a per-pair load would move).  Compute balance stays perfect
(8 pairs per core).

Device side (the heavy 1.37e11 FLOPs): per pair, two matmul layers in
transposed layout, contraction on the partition axis:
    hT[h,s]  = gelu(sum_d w1[d,h] * xT[d,s] + b1[h])     (16 h-tiles x 4 k-mm)
    eoT[o,s] = gelu(sum_h w2[h,o] * hT[h,s] + b2[o])     (4 o-tiles x 16 k-mm)
Weights/acts run through the PE in float16 (fp32 PSUM accumulation); biases
are fp32.  Outputs are written fp16 (halves the output DMA; ~5e-4 rel err
contribution).  Host combines in fp32: out[b] = (g0*eoT0 + g1*eoT1)^T.
"""

import os

import numpy as np

import jax

jax.config.update(
    "jax_compilation_cache_dir", os.path.expanduser("~/.jax_bass_cache")
)
jax.config.update("jax_persistent_cache_min_compile_time_secs", 0)
jax.config.update("jax_persistent_cache_min_entry_size_bytes", 0)

import concourse.bacc as bacc
import concourse.mybir as mybir
import concourse.tile as tile
from concourse.bass_utils import run_bass_kernel_spmd

B, S, D = 32, 512, 512
E, H, O, K = 8, 2048, 512, 2
NCORES = 8
PAIRS = (B * K) // NCORES  # 8 (batch, expert) pairs per core
# Uniform per-core expert-slot patterns, tried in order of weight-DMA cost:
# (6,1,1) loads 3 experts/core (12.6 MB) but needs sum(floor(n_e/6)) >= 8;
# (3,3,1,1) loads 4 (16.8 MB) and is ALWAYS feasible since
# sum(floor(n_e/3)) >= (64 - 2*8)/3 = 16.
SLOT_PATTERNS = ((6, 1, 1), (3, 3, 1, 1))

# PE dtype: float16 -- same 1 cyc/row speed as bf16, but a 10-bit
# mantissa (~4x less rounding error).  All values here are small (|x|<6,
# |w|<0.2, |z|<3), so fp16 range is not a concern.
MM_DT = mybir.dt.float16
NP_MM_DT = np.float16
F32 = mybir.dt.float32

DT_TILES = D // 128   # 4 k-tiles for layer 1
HT_TILES = H // 128   # 16 h-tiles
OT_TILES = O // 128   # 4 o-tiles

_nc_cache: dict = {}


def _build(repeat: int = 1, sizes: tuple = (3, 3, 1, 1)):
    """Build + compile the per-core SPMD program (identical on all cores).

    sizes is the uniform expert-slot pattern (pairs per slot, sum 8).
    repeat > 1 wraps the whole body in a hardware loop -- used only for
    timing (the body is idempotent)."""
    key = (repeat, sizes)
    if key in _nc_cache:
        return _nc_cache[key]
    nslots = len(sizes)

    nc = bacc.Bacc(
        "TRN2", target_bir_lowering=False, debug=False, num_devices=NCORES
    )
    xT_d = nc.dram_tensor("xT", [PAIRS, D, S], MM_DT, kind="ExternalInput")
    w1_d = nc.dram_tensor("w1u", [nslots, D, H], MM_DT, kind="ExternalInput")
    w2_d = nc.dram_tensor("w2u", [nslots, H, O], MM_DT, kind="ExternalInput")
    b1_d = nc.dram_tensor(
        "b1u", [nslots, 128, HT_TILES], F32, kind="ExternalInput"
    )
    b2_d = nc.dram_tensor(
        "b2u", [nslots, 128, OT_TILES], F32, kind="ExternalInput"
    )
    out_d = nc.dram_tensor("outT", [PAIRS, O, S], MM_DT, kind="ExternalOutput")

    with tile.TileContext(nc) as tc:
        with (
            tc.tile_pool(name="xp", bufs=4) as xp,
            tc.tile_pool(name="w1p", bufs=2) as w1p,
            tc.tile_pool(name="w2p", bufs=2) as w2p,
            tc.tile_pool(name="bp", bufs=2) as bp,
            tc.tile_pool(name="hp", bufs=3) as hp,
            tc.tile_pool(name="op", bufs=3) as op,
            tc.tile_pool(name="ps", bufs=8, space="PSUM") as psp,
        ):

            def slot_body(s, pair_base):
                # one expert's weights, loaded once, used by SLOT_SIZES[s]
                # pairs
                w1t = w1p.tile([128, DT_TILES, H], MM_DT)
                w1src = w1_d[s].rearrange("(t q) h -> q t h", q=128)
                for hc in range(4):
                    hs = slice(hc * (H // 4), (hc + 1) * (H // 4))
                    nc.sync.dma_start(w1t[:, :, hs], w1src[:, :, hs])
                w2t = w2p.tile([128, HT_TILES, O], MM_DT)
                w2src = w2_d[s].rearrange("(t q) o -> q t o", q=128)
                for tc2 in range(2):
                    ts2 = slice(
                        tc2 * (HT_TILES // 2), (tc2 + 1) * (HT_TILES // 2)
                    )
                    nc.sync.dma_start(w2t[:, ts2, :], w2src[:, ts2, :])
                b1t = bp.tile([128, HT_TILES], F32, tag="b1")
                nc.sync.dma_start(b1t[:], b1_d[s])
                b2t = bp.tile([128, OT_TILES], F32, tag="b2")
                nc.sync.dma_start(b2t[:], b2_d[s])

                for j in range(sizes[s]):
                    p = pair_base + j
                    xt = xp.tile([128, DT_TILES, S], MM_DT)
                    nc.sync.dma_start(
                        xt[:], xT_d[p].rearrange("(t q) s -> q t s", q=128)
                    )

                    ht = hp.tile([128, HT_TILES, S], MM_DT)
                    for t in range(HT_TILES):
                        ps = psp.tile([128, S], F32, tag="ps")
                        for d in range(DT_TILES):
                            nc.tensor.matmul(
                                ps[:],
                                w1t[:, d, t * 128 : (t + 1) * 128],
                                xt[:, d, :],
                                start=(d == 0),
                                stop=(d == DT_TILES - 1),
                            )
                        nc.scalar.activation(
                            ht[:, t, :],
                            ps[:],
                            mybir.ActivationFunctionType.Gelu,
                            bias=b1t[:, t : t + 1],
                        )

                    ot = op.tile([128, OT_TILES, S], MM_DT)
                    for o in range(OT_TILES):
                        ps = psp.tile([128, S], F32, tag="ps")
                        for t in range(HT_TILES):
                            nc.tensor.matmul(
                                ps[:],
                                w2t[:, t, o * 128 : (o + 1) * 128],
                                ht[:, t, :],
                                start=(t == 0),
                                stop=(t == HT_TILES - 1),
                            )
                        nc.scalar.activation(
                            ot[:, o, :],
                            ps[:],
                            mybir.ActivationFunctionType.Gelu,
                            bias=b2t[:, o : o + 1],
                        )
                    nc.sync.dma_start(
                        out_d[p].rearrange("(t q) s -> q t s", q=128), ot[:]
                    )

            def body():
                pair_base = 0
                for s in range(nslots):
                    slot_body(s, pair_base)
                    pair_base += sizes[s]

            if repeat == 1:
                body()
            else:
                with tc.For_i(
                    0,
                    repeat,
                    1,
                    staggered_reset=True,
                    hint_engines=(mybir.EngineType.PE,),
                ):
                    body()

    nc.compile()
    _nc_cache[key] = nc
    return nc


def _gating(x, attn_w, attn_b, gate_w, gate_b):
    """fp32 gating, op-for-op with the reference. Returns (idx [B,K], gn [B,K])."""
    f32 = np.float32
    x = x.astype(f32, copy=False)
    scores = x @ attn_w.astype(f32) + attn_b.astype(f32)          # [B,S,1]
    scores = scores - scores.max(axis=1, keepdims=True)
    e = np.exp(scores)
    aw = e / e.sum(axis=1, keepdims=True)
    pooled = (x * aw).sum(axis=1)                                  # [B,D]
    logits = pooled @ gate_w.astype(f32) + gate_b.astype(f32)      # [B,E]
    logits = logits - logits.max(axis=-1, keepdims=True)
    ge = np.exp(logits)
    gates = ge / ge.sum(axis=-1, keepdims=True)
    # top-k with lower-index tie-break, like lax.top_k
    idx = np.argsort(-gates, axis=-1, kind="stable")[:, :K]        # [B,K]
    gg = np.take_along_axis(gates, idx, axis=-1)
    gn = gg / (gg.sum(axis=-1, keepdims=True) + f32(1e-9))
    return idx, gn


def _schedule(idx, gn):
    """Decompose the 64 (b, e, g) pairs into mono-expert parts matching a
    uniform per-core slot pattern, preferring the pattern with the fewest
    weight loads.  A pattern (big, ..., 1, ..., 1) with nbig big-parts and
    nones 1-parts per core is feasible iff sum(floor(n_e/big)) >= 8*nbig;
    (3,3,1,1) is always feasible.

    Returns, per core, a list of (expert, [(b, g), ...]) with the uniform
    slot sizes."""
    by_expert: dict = {}
    for b in range(B):
        for k in range(K):
            by_expert.setdefault(int(idx[b, k]), []).append(
                (b, float(gn[b, k]))
            )
    experts = sorted(by_expert)

    for sizes in SLOT_PATTERNS:
        big = sizes[0]
        nbig = sum(1 for s in sizes if s == big)
        nones = sum(1 for s in sizes if s == 1)
        assert sizes == (big,) * nbig + (1,) * nones
        need_big = NCORES * nbig
        bigs = {e: len(by_expert[e]) // big for e in experts}
        ones = {e: len(by_expert[e]) % big for e in experts}
        excess = sum(bigs.values()) - need_big
        if excess < 0:
            continue  # pattern infeasible for this routing
        for e in sorted(experts, key=lambda e: -bigs[e]):
            while excess > 0 and bigs[e] > 0:
                bigs[e] -= 1
                ones[e] += big
                excess -= 1
        assert sum(bigs.values()) == need_big
        assert sum(ones.values()) == NCORES * nones

        parts_big, parts_1 = [], []
        for e in experts:
            plist = by_expert[e]
            pos = 0
            for _ in range(bigs[e]):
                parts_big.append((e, plist[pos : pos + big]))
                pos += big
            for _ in range(ones[e]):
                parts_1.append((e, plist[pos : pos + 1]))
                pos += 1
            assert pos == len(plist)

        sched = []
        for c in range(NCORES):
            sched.append(
                parts_big[nbig * c : nbig * (c + 1)]
                + parts_1[nones * c : nones * (c + 1)]
            )
        return sched
    raise AssertionError("unreachable: (3,3,1,1) is always feasible")


def _make_in_maps(x, w1, b1, w2, b2, sched):
    """Per-core input dicts for the slot-structured program."""
    w1_c = np.ascontiguousarray(w1).astype(NP_MM_DT)               # [E,D,H]
    w2_c = np.ascontiguousarray(w2).astype(NP_MM_DT)               # [E,H,O]
    xT_c = np.ascontiguousarray(
        np.asarray(x).transpose(0, 2, 1)
    ).astype(NP_MM_DT)                                             # [B,D,S]
    b1_t = np.ascontiguousarray(
        np.asarray(b1).reshape(E, HT_TILES, 128).transpose(0, 2, 1)
    ).astype(np.float32)                                           # [E,128,16]
    b2_t = np.ascontiguousarray(
        np.asarray(b2).reshape(E, OT_TILES, 128).transpose(0, 2, 1)
    ).astype(np.float32)                                           # [E,128,4]

    in_maps = []
    for c in range(NCORES):
        es = [slot[0] for slot in sched[c]]
        bs = [b for slot in sched[c] for (b, g) in slot[1]]
        in_maps.append(
            {
                "xT": xT_c[bs],
                "w1u": w1_c[es],
                "w2u": w2_c[es],
                "b1u": b1_t[es],
                "b2u": b2_t[es],
            }
        )
    return in_maps


def kernel(
    x, attn_w, attn_b, gate_w, gate_b, w1, b1, w2, b2
) -> np.ndarray:
    x = np.asarray(x)
    idx, gn = _gating(
        x, np.asarray(attn_w), np.asarray(attn_b), np.asarray(gate_w),
        np.asarray(gate_b),
    )
    sched = _schedule(idx, gn)
    in_maps = _make_in_maps(
        x, np.asarray(w1), np.asarray(b1), np.asarray(w2), np.asarray(b2),
        sched,
    )

    sizes = tuple(len(slot[1]) for slot in sched[0])
    nc = _build(repeat=1, sizes=sizes)
    br = run_bass_kernel_spmd(nc, in_maps, list(range(NCORES)))

    out = np.zeros((B, S, O), np.float32)
    for c in range(NCORES):
        eoT = br.results[c]["outT"].astype(np.float32)             # [PAIRS,O,S]
        pairs = [(b, g) for slot in sched[c] for (b, g) in slot[1]]
        for p, (b, g) in enumerate(pairs):
            out[b] += np.float32(g) * eoT[p].T
    return out


# revision 14
# speedup vs baseline: 1.0813x; 1.0813x over previous
"""Trainium2 Bass kernel for a top-2-of-8 MoE layer (attention-pooled gating).

Strategy
--------
The reference computes every expert densely and combines with weights ``g``
that have exactly K=2 nonzeros per batch (softmax -> top-k mask -> renorm).
So the mathematically identical computation is: route each batch to its top-2
experts and compute only those 64 (batch, expert) pairs.

Host side (cheap, O(B*S*D)): attention-pool gating in fp32 mirroring the
reference op-for-op, top-2 selection, renormalized weights.

Scheduling: the 64 pairs are decomposed into 16 mono-expert parts of 3 pairs
and 16 mono-expert parts of 1 pair (always feasible: sum(floor(n_e/3)) >=
(64 - 2*8)/3 = 16), and every core gets two 3-parts and two 1-parts -- a
uniform [3, 3, 1, 1] slot pattern, so one SPMD program serves any routing
while loading each slot's expert weights ONCE (16.8 MB/core instead of the
33.6 MB/core a per-pair load would move).  Compute balance stays perfect
(8 pairs per core).

Device side (the heavy 1.37e11 FLOPs): per pair, two matmul layers in
transposed layout, contraction on the partition axis:
    hT[h,s]  = gelu(sum_d w1[d,h] * xT[d,s] + b1[h])     (16 h-tiles x 4 k-mm)
    eoT[o,s] = gelu(sum_h w2[h,o] * hT[h,s] + b2[o])     (4 o-tiles x 16 k-mm)
Weights/acts run through the PE in float16 (fp32 PSUM accumulation); biases
are fp32.  Outputs are written fp16 (halves the output DMA; ~5e-4 rel err
contribution).  Host combines in fp32: out[b] = (g0*eoT0 + g1*eoT1)^T.
"""

import os

import numpy as np

import jax

jax.config.update(
    "jax_compilation_cache_dir", os.path.expanduser("~/.jax_bass_cache")
)
jax.config.update("jax_persistent_cache_min_compile_time_secs", 0)
jax.config.update("jax_persistent_cache_min_entry_size_bytes", 0)

import concourse.bacc as bacc
import concourse.mybir as mybir
import concourse.tile as tile
from concourse.bass_utils import run_bass_kernel_spmd

B, S, D = 32, 512, 512
E, H, O, K = 8, 2048, 512, 2
NCORES = 8
PAIRS = (B * K) // NCORES  # 8 (batch, expert) pairs per core
# Uniform per-core expert-slot patterns, tried in order of weight-DMA cost:
# (6,1,1) loads 3 experts/core (12.6 MB) but needs sum(floor(n_e/6)) >= 8;
# (3,3,1,1) loads 4 (16.8 MB) and is ALWAYS feasible since
# sum(floor(n_e/3)) >= (64 - 2*8)/3 = 16.
SLOT_PATTERNS = ((6, 1, 1), (3, 3, 1, 1))

import ml_dtypes

# PE dtype: bfloat16.  Same documented 1 cyc/row as fp16, but measured
# 7-9% faster SUSTAINED on this thermally-throttled part (8/8 interleaved
# rounds): the 8-bit-mantissa multiplier draws less array power than
# fp16's 11-bit, so the power-governed clock settles higher.  End-to-end
# error 4.1e-3 vs the 2e-2 gate (fp16 was 4.6e-4 -- both pass; bf16 is
# the better speed/accuracy point here).
MM_DT = mybir.dt.bfloat16
NP_MM_DT = ml_dtypes.bfloat16
F32 = mybir.dt.float32

DT_TILES = D // 128   # 4 k-tiles for layer 1
HT_TILES = H // 128   # 16 h-tiles
OT_TILES = O // 128   # 4 o-tiles

_nc_cache: dict = {}


def _build(repeat: int = 1, sizes: tuple = (3, 3, 1, 1)):
    """Build + compile the per-core SPMD program (identical on all cores).

    sizes is the uniform expert-slot pattern (pairs per slot, sum 8).
    repeat > 1 wraps the whole body in a hardware loop -- used only for
    timing (the body is idempotent)."""
    key = (repeat, sizes)
    if key in _nc_cache:
        return _nc_cache[key]
    nslots = len(sizes)

    nc = bacc.Bacc(
        "TRN2", target_bir_lowering=False, debug=False, num_devices=NCORES
    )
    xT_d = nc.dram_tensor("xT", [PAIRS, D, S], MM_DT, kind="ExternalInput")
    w1_d = nc.dram_tensor("w1u", [nslots, D, H], MM_DT, kind="ExternalInput")
    w2_d = nc.dram_tensor("w2u", [nslots, H, O], MM_DT, kind="ExternalInput")
    b1_d = nc.dram_tensor(
        "b1u", [nslots, 128, HT_TILES], F32, kind="ExternalInput"
    )
    b2_d = nc.dram_tensor(
        "b2u", [nslots, 128, OT_TILES], F32, kind="ExternalInput"
    )
    out_d = nc.dram_tensor("outT", [PAIRS, O, S], MM_DT, kind="ExternalOutput")

    with tile.TileContext(nc) as tc:
        with (
            tc.tile_pool(name="xp", bufs=4) as xp,
            tc.tile_pool(name="w1p", bufs=2) as w1p,
            tc.tile_pool(name="w2p", bufs=2) as w2p,
            tc.tile_pool(name="bp", bufs=2) as bp,
            tc.tile_pool(name="hp", bufs=3) as hp,
            tc.tile_pool(name="op", bufs=3) as op,
            tc.tile_pool(name="ps", bufs=8, space="PSUM") as psp,
        ):

            def slot_body(s, pair_base):
                # one expert's weights, loaded once, used by SLOT_SIZES[s]
                # pairs
                w1t = w1p.tile([128, DT_TILES, H], MM_DT)
                w1src = w1_d[s].rearrange("(t q) h -> q t h", q=128)
                for hc in range(4):
                    hs = slice(hc * (H // 4), (hc + 1) * (H // 4))
                    nc.sync.dma_start(w1t[:, :, hs], w1src[:, :, hs])
                w2t = w2p.tile([128, HT_TILES, O], MM_DT)
                w2src = w2_d[s].rearrange("(t q) o -> q t o", q=128)
                for tc2 in range(2):
                    ts2 = slice(
                        tc2 * (HT_TILES // 2), (tc2 + 1) * (HT_TILES // 2)
                    )
                    nc.sync.dma_start(w2t[:, ts2, :], w2src[:, ts2, :])
                b1t = bp.tile([128, HT_TILES], F32, tag="b1")
                nc.sync.dma_start(b1t[:], b1_d[s])
                b2t = bp.tile([128, OT_TILES], F32, tag="b2")
                nc.sync.dma_start(b2t[:], b2_d[s])

                for j in range(sizes[s]):
                    p = pair_base + j
                    xt = xp.tile([128, DT_TILES, S], MM_DT)
                    nc.sync.dma_start(
                        xt[:], xT_d[p].rearrange("(t q) s -> q t s", q=128)
                    )

                    ht = hp.tile([128, HT_TILES, S], MM_DT)
                    for t in range(HT_TILES):
                        ps = psp.tile([128, S], F32, tag="ps")
                        for d in range(DT_TILES):
                            nc.tensor.matmul(
                                ps[:],
                                w1t[:, d, t * 128 : (t + 1) * 128],
                                xt[:, d, :],
                                start=(d == 0),
                                stop=(d == DT_TILES - 1),
                            )
                        nc.scalar.activation(
                            ht[:, t, :],
                            ps[:],
                            mybir.ActivationFunctionType.Gelu,
                            bias=b1t[:, t : t + 1],
                        )

                    ot = op.tile([128, OT_TILES, S], MM_DT)
                    for o in range(OT_TILES):
                        ps = psp.tile([128, S], F32, tag="ps")
                        for t in range(HT_TILES):
                            nc.tensor.matmul(
                                ps[:],
                                w2t[:, t, o * 128 : (o + 1) * 128],
                                ht[:, t, :],
                                start=(t == 0),
                                stop=(t == HT_TILES - 1),
                            )
                        nc.scalar.activation(
                            ot[:, o, :],
                            ps[:],
                            mybir.ActivationFunctionType.Gelu,
                            bias=b2t[:, o : o + 1],
                        )
                    nc.sync.dma_start(
                        out_d[p].rearrange("(t q) s -> q t s", q=128), ot[:]
                    )

            def body():
                pair_base = 0
                for s in range(nslots):
                    slot_body(s, pair_base)
                    pair_base += sizes[s]

            if repeat == 1:
                body()
            else:
                with tc.For_i(
                    0,
                    repeat,
                    1,
                    staggered_reset=True,
                    hint_engines=(mybir.EngineType.PE,),
                ):
                    body()

    nc.compile()
    _nc_cache[key] = nc
    return nc


def _gating(x, attn_w, attn_b, gate_w, gate_b):
    """fp32 gating, op-for-op with the reference. Returns (idx [B,K], gn [B,K])."""
    f32 = np.float32
    x = x.astype(f32, copy=False)
    scores = x @ attn_w.astype(f32) + attn_b.astype(f32)          # [B,S,1]
    scores = scores - scores.max(axis=1, keepdims=True)
    e = np.exp(scores)
    aw = e / e.sum(axis=1, keepdims=True)
    pooled = (x * aw).sum(axis=1)                                  # [B,D]
    logits = pooled @ gate_w.astype(f32) + gate_b.astype(f32)      # [B,E]
    logits = logits - logits.max(axis=-1, keepdims=True)
    ge = np.exp(logits)
    gates = ge / ge.sum(axis=-1, keepdims=True)
    # top-k with lower-index tie-break, like lax.top_k
    idx = np.argsort(-gates, axis=-1, kind="stable")[:, :K]        # [B,K]
    gg = np.take_along_axis(gates, idx, axis=-1)
    gn = gg / (gg.sum(axis=-1, keepdims=True) + f32(1e-9))
    return idx, gn


def _schedule(idx, gn):
    """Decompose the 64 (b, e, g) pairs into mono-expert parts matching a
    uniform per-core slot pattern, preferring the pattern with the fewest
    weight loads.  A pattern (big, ..., 1, ..., 1) with nbig big-parts and
    nones 1-parts per core is feasible iff sum(floor(n_e/big)) >= 8*nbig;
    (3,3,1,1) is always feasible.

    Returns, per core, a list of (expert, [(b, g), ...]) with the uniform
    slot sizes."""
    by_expert: dict = {}
    for b in range(B):
        for k in range(K):
            by_expert.setdefault(int(idx[b, k]), []).append(
                (b, float(gn[b, k]))
            )
    experts = sorted(by_expert)

    for sizes in SLOT_PATTERNS:
        big = sizes[0]
        nbig = sum(1 for s in sizes if s == big)
        nones = sum(1 for s in sizes if s == 1)
        assert sizes == (big,) * nbig + (1,) * nones
        need_big = NCORES * nbig
        bigs = {e: len(by_expert[e]) // big for e in experts}
        ones = {e: len(by_expert[e]) % big for e in experts}
        excess = sum(bigs.values()) - need_big
        if excess < 0:
            continue  # pattern infeasible for this routing
        for e in sorted(experts, key=lambda e: -bigs[e]):
            while excess > 0 and bigs[e] > 0:
                bigs[e] -= 1
                ones[e] += big
                excess -= 1
        assert sum(bigs.values()) == need_big
        assert sum(ones.values()) == NCORES * nones

        parts_big, parts_1 = [], []
        for e in experts:
            plist = by_expert[e]
            pos = 0
            for _ in range(bigs[e]):
                parts_big.append((e, plist[pos : pos + big]))
                pos += big
            for _ in range(ones[e]):
                parts_1.append((e, plist[pos : pos + 1]))
                pos += 1
            assert pos == len(plist)

        sched = []
        for c in range(NCORES):
            sched.append(
                parts_big[nbig * c : nbig * (c + 1)]
                + parts_1[nones * c : nones * (c + 1)]
            )
        return sched
    raise AssertionError("unreachable: (3,3,1,1) is always feasible")


def _make_in_maps(x, w1, b1, w2, b2, sched):
    """Per-core input dicts for the slot-structured program."""
    w1_c = np.ascontiguousarray(w1).astype(NP_MM_DT)               # [E,D,H]
    w2_c = np.ascontiguousarray(w2).astype(NP_MM_DT)               # [E,H,O]
    xT_c = np.ascontiguousarray(
        np.asarray(x).transpose(0, 2, 1)
    ).astype(NP_MM_DT)                                             # [B,D,S]
    b1_t = np.ascontiguousarray(
        np.asarray(b1).reshape(E, HT_TILES, 128).transpose(0, 2, 1)
    ).astype(np.float32)                                           # [E,128,16]
    b2_t = np.ascontiguousarray(
        np.asarray(b2).reshape(E, OT_TILES, 128).transpose(0, 2, 1)
    ).astype(np.float32)                                           # [E,128,4]

    in_maps = []
    for c in range(NCORES):
        es = [slot[0] for slot in sched[c]]
        bs = [b for slot in sched[c] for (b, g) in slot[1]]
        in_maps.append(
            {
                "xT": xT_c[bs],
                "w1u": w1_c[es],
                "w2u": w2_c[es],
                "b1u": b1_t[es],
                "b2u": b2_t[es],
            }
        )
    return in_maps


def kernel(
    x, attn_w, attn_b, gate_w, gate_b, w1, b1, w2, b2
) -> np.ndarray:
    x = np.asarray(x)
    idx, gn = _gating(
        x, np.asarray(attn_w), np.asarray(attn_b), np.asarray(gate_w),
        np.asarray(gate_b),
    )
    sched = _schedule(idx, gn)
    in_maps = _make_in_maps(
        x, np.asarray(w1), np.asarray(b1), np.asarray(w2), np.asarray(b2),
        sched,
    )

    sizes = tuple(len(slot[1]) for slot in sched[0])
    nc = _build(repeat=1, sizes=sizes)
    br = run_bass_kernel_spmd(nc, in_maps, list(range(NCORES)))

    out = np.zeros((B, S, O), np.float32)
    for c in range(NCORES):
        eoT = br.results[c]["outT"].astype(np.float32)             # [PAIRS,O,S]
        pairs = [(b, g) for slot in sched[c] for (b, g) in slot[1]]
        for p, (b, g) in enumerate(pairs):
            out[b] += np.float32(g) * eoT[p].T
    return out
